# revision 6
# baseline (speedup 1.0000x reference)
"""Multi-head attention (B=2, S=2048, D=1024, H=16, dk=64) on 8 TRN2 cores.

Sharding: core c -> (batch b = c//4, head-group g = c%4 of 4 heads).
Each core computes q/k/v projections for its 4 heads, full attention for
those heads, and a partial output projection (rows g*256:(g+1)*256 of Wo).
Host pre-transposes/casts inputs to bf16 and sums the partial outputs.

Device layout (per core, all matmul operands bf16, accumulation f32):
  xqT/xkT/xvT [1024, 2048]   (d on partitions -> contraction-ready)
  qT, kT      [256, 2048]    (head-dim on partitions; pair tiles [128, S])
  v_aug       [2048, 4*65]   (per head: [v_h | ones]; ones col => softmax denom)
  scoresT     [j, i] in PSUM; exp on ScalarE -> probsT bf16 (no max-subtract:
              scores ~ N(0,1) after 1/8 scaling, exp bounded ~e^6)
  PV:         attnT_unnorm[e, i] = sum_j v_aug[j, e] * probsT[j, i]
              (row 64 = softmax denominator), normalize via reciprocal +
              K=1 broadcast matmul, store attnT [64, S] per head
  out-projT:  outT[n, s] = sum_{h,e} wo[h,e,n] * attnT_h[e, s]  (K=64 x4)
Host: out[b] = sum_g outT_partial.T + (bv @ Wo + bo).
"""

import os

import numpy as np
import ml_dtypes

BF16 = ml_dtypes.bfloat16

B, S, D = 2, 2048, 1024
H, DK = 16, 64
P = 128
GROUPS = 4          # head groups (one per core within a batch)
HPG = 4             # heads per group
GD = HPG * DK       # 256, group width
KC = D // P         # 8 contraction chunks
ST = S // P         # 16 s-tiles / j-tiles
NCORES = 8

_cached = {}


def _build_bass():
    import concourse.bass as bass
    import concourse.tile as tile
    from concourse.bacc import Bacc
    from concourse import mybir
    from contextlib import ExitStack

    f32 = mybir.dt.float32
    bf16 = mybir.dt.bfloat16
    Act = mybir.ActivationFunctionType

    nc = Bacc()

    xqT = nc.dram_tensor("xqT", [D, S], bf16, kind="ExternalInput")
    xkT = nc.dram_tensor("xkT", [D, S], bf16, kind="ExternalInput")
    xvT = nc.dram_tensor("xvT", [D, S], bf16, kind="ExternalInput")
    wq = nc.dram_tensor("wq", [D, GD], bf16, kind="ExternalInput")
    wk = nc.dram_tensor("wk", [D, GD], bf16, kind="ExternalInput")
    wv = nc.dram_tensor("wv", [D, GD], bf16, kind="ExternalInput")
    wo = nc.dram_tensor("wo", [GD, D], bf16, kind="ExternalInput")
    bq = nc.dram_tensor("bq", [GD, 1], f32, kind="ExternalInput")
    bk = nc.dram_tensor("bk", [GD, 1], f32, kind="ExternalInput")
    outT = nc.dram_tensor("outT", [D, S], f32, kind="ExternalOutput")

    with tile.TileContext(nc) as tc, ExitStack() as ctx:
        singles = ctx.enter_context(tc.tile_pool(name="singles", bufs=1))
        probs_pool = ctx.enter_context(tc.tile_pool(name="probs", bufs=3))
        small = ctx.enter_context(tc.tile_pool(name="small", bufs=2))
        outs_pool = ctx.enter_context(tc.tile_pool(name="outs", bufs=4))
        psum = ctx.enter_context(tc.tile_pool(name="psum", bufs=1, space="PSUM"))

        # ---- persistent SBUF ----
        wq_sb = singles.tile([P, KC, GD], bf16)
        wk_sb = singles.tile([P, KC, GD], bf16)
        wv_sb = singles.tile([P, KC, GD], bf16)
        wo_sb = singles.tile([DK, HPG, D], bf16)
        bq_sb = singles.tile([P, 2, 1], f32)
        bk_sb = singles.tile([P, 2, 1], f32)
        nc.sync.dma_start(out=wq_sb, in_=wq.rearrange("(c p) m -> p c m", p=P))
        nc.sync.dma_start(out=wk_sb, in_=wk.rearrange("(c p) m -> p c m", p=P))
        nc.sync.dma_start(out=wv_sb, in_=wv.rearrange("(c p) m -> p c m", p=P))
        nc.sync.dma_start(out=wo_sb, in_=wo.rearrange("(h p) n -> p h n", p=DK))
        nc.sync.dma_start(out=bq_sb, in_=bq.rearrange("(t p) o -> p t o", p=P))
        nc.sync.dma_start(out=bk_sb, in_=bk.rearrange("(t p) o -> p t o", p=P))

        xq_sb = singles.tile([P, KC, S], bf16)
        xk_sb = singles.tile([P, KC, S], bf16)
        xv_sb = singles.tile([P, KC, S], bf16)
        for k in range(KC):
            nc.sync.dma_start(out=xq_sb[:, k, :], in_=xqT[k * P:(k + 1) * P, :])
            nc.sync.dma_start(out=xk_sb[:, k, :], in_=xkT[k * P:(k + 1) * P, :])
            nc.sync.dma_start(out=xv_sb[:, k, :], in_=xvT[k * P:(k + 1) * P, :])

        qT_sb = [singles.tile([P, S], bf16, name=f"qT{t}") for t in range(2)]
        kT_sb = [singles.tile([P, S], bf16, name=f"kT{t}") for t in range(2)]
        v_sb = singles.tile([P, ST, HPG * 65], bf16)
        att_sb = [singles.tile([DK, S], bf16, name=f"att{h}") for h in range(HPG)]

        ones_sb = singles.tile([65, DK], f32)
        nc.vector.memset(ones_sb[64:65, :], 1.0)

        # ones columns of v_aug (col 64 of each per-head [64|1] block)
        v4 = v_sb.rearrange("p s (h c) -> p s h c", c=65)
        nc.vector.memset(v4[:, :, :, 64:65], 1.0)

        # ---- phase A: projections ----
        def qk_proj(x_sb, w_sb, b_sb, dst, t):
            pq = [psum.tile([P, 1024], mybir.dt.float32, tag="sc", bufs=2,
                            name=f"pq{t}{half}") for half in range(2)]
            for k in range(KC):
                for half in range(2):
                    for sq in range(2):
                        nc.tensor.matmul(
                            out=pq[half][:, sq * 512:(sq + 1) * 512],
                            lhsT=w_sb[:, k, t * P:(t + 1) * P],
                            rhs=x_sb[:, k, half * 1024 + sq * 512:
                                     half * 1024 + (sq + 1) * 512],
                            start=(k == 0), stop=(k == KC - 1))
            for half in range(2):
                nc.vector.tensor_scalar_add(
                    out=dst[:, half * 1024:(half + 1) * 1024],
                    in0=pq[half], scalar1=b_sb[:, t, :])

        def v_proj():
            for st in range(ST):
                pvv = psum.tile([P, GD], mybir.dt.float32, tag="pv", bufs=4, name="pvv")
                for k in range(KC):
                    nc.tensor.matmul(
                        out=pvv,
                        lhsT=xv_sb[:, k, st * P:(st + 1) * P],
                        rhs=wv_sb[:, k, :],
                        start=(k == 0), stop=(k == KC - 1))
                dst = v4[:, st, :, 0:64]
                src = pvv.rearrange("p (h c) -> p h c", c=64)
                nc.vector.tensor_copy(out=dst, in_=src)

        # ---- phase B: attention for one head pair, one i-half ----
        def attention(pair, ih):
            pv = [[psum.tile([65, 512], mybir.dt.float32, tag="pv", bufs=4,
                             name=f"pv{pair}{ih}{hp}{iq}")
                   for iq in range(2)] for hp in range(2)]
            for jt in range(ST):
                sc = [psum.tile([P, 1024], mybir.dt.float32, tag="sc", bufs=2,
                                name=f"sc{hp}") for hp in range(2)]
                for iq in range(2):
                    for hp in range(2):
                        nc.tensor.matmul(
                            out=sc[hp][:, iq * 512:(iq + 1) * 512],
                            lhsT=kT_sb[pair][hp * 64:(hp + 1) * 64,
                                             jt * P:(jt + 1) * P],
                            rhs=qT_sb[pair][hp * 64:(hp + 1) * 64,
                                            ih * 1024 + iq * 512:
                                            ih * 1024 + (iq + 1) * 512],
                            start=True, stop=True)
                for hp in range(2):
                    probs = probs_pool.tile([P, 1024], bf16, tag="probs",
                                            name="probs")
                    nc.scalar.activation(out=probs, in_=sc[hp], func=Act.Exp,
                                         scale=0.125)
                    h65 = (2 * pair + hp) * 65
                    for iq in range(2):
                        nc.tensor.matmul(
                            out=pv[hp][iq][:, :],
                            lhsT=v_sb[:, jt, h65:h65 + 65],
                            rhs=probs[:, iq * 512:(iq + 1) * 512],
                            start=(jt == 0), stop=(jt == ST - 1))
            # normalize + store attnT
            for hp in range(2):
                h = 2 * pair + hp
                for iq in range(2):
                    r = small.tile([65, 512], mybir.dt.float32, tag="r",
                                   name="r")
                    nc.vector.reciprocal(out=r[64:65, :],
                                         in_=pv[hp][iq][64:65, :])
                    bc = psum.tile([64, 512], mybir.dt.float32, tag="sc", bufs=2,
                                   name="bc")
                    nc.tensor.matmul(out=bc, lhsT=ones_sb[64:65, :],
                                     rhs=r[64:65, :], start=True, stop=True)
                    pvs = small.tile([64, 512], mybir.dt.float32, tag="pvs",
                                     name="pvs")
                    nc.vector.tensor_copy(out=pvs, in_=pv[hp][iq][0:64, :])
                    col = ih * 1024 + iq * 512
                    nc.vector.tensor_mul(
                        out=att_sb[h][:, col:col + 512],
                        in0=pvs, in1=bc)

        def out_proj():
            for nt in range(KC):
                po = [psum.tile([P, 512], mybir.dt.float32, tag="pv", bufs=4,
                                name=f"po{sb4}") for sb4 in range(4)]
                for h in range(HPG):
                    for sb4 in range(4):
                        nc.tensor.matmul(
                            out=po[sb4],
                            lhsT=wo_sb[:, h, nt * P:(nt + 1) * P],
                            rhs=att_sb[h][:, sb4 * 512:(sb4 + 1) * 512],
                            start=(h == 0), stop=(h == HPG - 1))
                for sb4 in range(4):
                    osb = outs_pool.tile([P, 512], mybir.dt.float32,
                                         tag="osb", name="osb")
                    nc.vector.tensor_copy(out=osb, in_=po[sb4])
                    nc.sync.dma_start(
                        out=outT[nt * P:(nt + 1) * P,
                                 sb4 * 512:(sb4 + 1) * 512],
                        in_=osb)

        qk_proj(xq_sb, wq_sb, bq_sb, qT_sb[0], 0)
        qk_proj(xk_sb, wk_sb, bk_sb, kT_sb[0], 0)
        v_proj()
        qk_proj(xq_sb, wq_sb, bq_sb, qT_sb[1], 1)
        qk_proj(xk_sb, wk_sb, bk_sb, kT_sb[1], 1)
        for pair in range(2):
            for ih in range(2):
                attention(pair, ih)
        out_proj()

    nc.finalize()
    return nc


def kernel(Q, K, V, Wq, bq, Wk, bk, Wv, bv, Wo, bo):
    from concourse.bass_utils import run_bass_kernel_spmd

    f32 = np.float32
    Q = np.asarray(Q, f32)
    K = np.asarray(K, f32)
    V = np.asarray(V, f32)
    Wq = np.asarray(Wq, f32)
    Wk = np.asarray(Wk, f32)
    Wv = np.asarray(Wv, f32)
    Wo = np.asarray(Wo, f32)
    bq = np.asarray(bq, f32)
    bk = np.asarray(bk, f32)
    bv = np.asarray(bv, f32)
    bo = np.asarray(bo, f32)

    xT = {}
    for b in range(B):
        xT[('q', b)] = np.ascontiguousarray(Q[b].T).astype(BF16)
        xT[('k', b)] = np.ascontiguousarray(K[b].T).astype(BF16)
        xT[('v', b)] = np.ascontiguousarray(V[b].T).astype(BF16)

    in_maps = []
    for c in range(NCORES):
        b, g = c // GROUPS, c % GROUPS
        sl = slice(g * GD, (g + 1) * GD)
        in_maps.append({
            "xqT": xT[('q', b)],
            "xkT": xT[('k', b)],
            "xvT": xT[('v', b)],
            "wq": np.ascontiguousarray(Wq[:, sl]).astype(BF16),
            "wk": np.ascontiguousarray(Wk[:, sl]).astype(BF16),
            "wv": np.ascontiguousarray(Wv[:, sl]).astype(BF16),
            "wo": np.ascontiguousarray(Wo[sl, :]).astype(BF16),
            "bq": np.ascontiguousarray(bq[sl].reshape(GD, 1)),
            "bk": np.ascontiguousarray(bk[sl].reshape(GD, 1)),
        })

    if "nc" not in _cached:
        _cached["nc"] = _build_bass()
    nc = _cached["nc"]

    res = run_bass_kernel_spmd(nc, in_maps, core_ids=list(range(NCORES)))
    if res.exec_time_ns is not None:
        print(f"HW exec time: {res.exec_time_ns} ns")

    bo_eff = (bv @ Wo + bo).astype(f32)
    out = np.zeros((B, S, D), f32)
    for c in range(NCORES):
        b = c // GROUPS
        out[b] += res.results[c]["outT"].T
    out += bo_eff
    return out
